# revision 27
# baseline (speedup 1.0000x reference)
"""BERT per-word mean-pool (segment reduce) on 8 Trainium2 NeuronCores.

Problem: output[B=64, S=512, E=768] f32, mappings[B, W=255] int32 (values 1 or 2).
Per sentence, strip [CLS]/[SEP], mean-pool contiguous BPE spans into word vectors.

Key identity: every word's span has 1 or 2 BPE rows.  With s = span start and
e = span end (exclusive) inside the stripped sequence, the mean is ALWAYS
    out[w] = (hs[s] + hs[e-1]) * 0.5
because for a 1-token span s == e-1 and (x + x)/2 == x exactly in f32.
So the whole kernel is two row-gathers, an add, and a scale by 0.5.

Sharding: pure data parallel, 8 sentences per core, no cross-core comms.
Device work per core: dma_gather 2x2048 rows of 3KB (12.6 MB), DVE add,
ACT scale, contiguous store (6.3 MB) -> memory-bound at ~360 GB/s.

Uses the InstDMAGatherAnt custom SWDGE gather (mlp ucode library,
auto-loaded by Bacc.compile) -- the production-proven gather path; raw
indirect InstDMACopy corrupts when two indirect DMAs are in flight.
"""

import numpy as np

from concourse import bacc, bass, mybir, tile
from concourse.bass_utils import run_bass_kernel_spmd

B, S, W, E = 64, 512, 255, 768
NCORES = 8
BPC = B // NCORES            # sentences per core
NW = BPC * W                 # 2040 real words per core
NWP = 2048                   # padded word count (multiple of 512)
NCHUNK = 4                   # chunks per core
CPW = NWP // NCHUNK          # 512 words per chunk
JJ = CPW // 128              # 4 words per partition per chunk
ROWS = BPC * S               # 4096 input rows per core
NIDX = 2 * CPW               # 1024 gather indices per chunk (A then B)

_F32 = mybir.dt.float32
_I16 = mybir.dt.int16


def _build_nc(reps=1, bufs=2, order="pc", nq=1, mode="ab", nchunk=NCHUNK,
              merged_idx=False, warm=False, hscale=False):
    nc = bacc.Bacc(
        "TRN2",
        target_bir_lowering=False,
        debug=False,
        num_devices=NCORES,
        num_swdge_queues=nq,
    )
    x = nc.dram_tensor("x", [ROWS, E], _F32, kind="ExternalInput").ap()
    # indices are int16, wrapped [i%16, i//16] into 16 partitions and
    # replicated 8x down to 128 partitions (Q7 core replication).
    cpw = NWP // nchunk
    jj = cpw // 128
    nidx = 2 * cpw if mode == "ab" else cpw
    idx = nc.dram_tensor(
        "idx", [nchunk, 128, nidx // 16], _I16, kind="ExternalInput"
    ).ap()
    if mode == "win":
        # per word w: rw[p, 2c] = 1/m(w), rw[p, 2c+1] = (m(w)-1)/m(w)
        rw = nc.dram_tensor(
            "rw", [nchunk, 128, 2 * jj], _F32, kind="ExternalInput"
        ).ap()
    y = nc.dram_tensor("y", [NWP, E], _F32, kind="ExternalOutput").ap()

    with tile.TileContext(nc) as tc:
        with (
            tc.tile_pool(name="idxp", bufs=1) as ipool,
            tc.tile_pool(name="io", bufs=bufs) as pool,
        ):
            if warm:
                # dummy 16-index gather issued first: triggers the Q7
                # ucode IRAM fetch (~6us) while the idx loads stream in,
                # so the first real gather isn't stalled on it.
                wi = ipool.tile([128, 1], _I16, tag="warmi")
                nc.gpsimd.memset(wi[:], 0)
                wo = ipool.tile([128, E], _F32, tag="warmo")
                nc.gpsimd.dma_gather(
                    wo[:].rearrange("p (c e) -> p c e", e=E),
                    x[:, :], wi[:], 16, 16, E,
                )
            its, rts = [], []
            if merged_idx:
                itall = ipool.tile([128, nchunk * (nidx // 16)], _I16, tag="itall")
                nc.sync.dma_start(
                    out=itall[:].rearrange("p (q s) -> p q s", q=nchunk),
                    in_=idx.rearrange("q p s -> p q s"),
                )
                its = [
                    itall[:, q * (nidx // 16) : (q + 1) * (nidx // 16)]
                    for q in range(nchunk)
                ]
            else:
                for q in range(nchunk):
                    it = ipool.tile([128, nidx // 16], _I16, tag=f"it{q}")
                    nc.sync.dma_start(out=it[:], in_=idx[q])
                    its.append(it[:])
            if mode == "win":
                for q in range(nchunk):
                    rt = ipool.tile([128, 2 * jj], _F32, tag=f"rt{q}")
                    nc.sync.dma_start(out=rt[:], in_=rw[q])
                    rts.append(rt)
            for _rep in range(reps):
                for q in range(nchunk):
                    if mode == "ab":
                        # gathered slot i -> T[i % 128, i // 128, :]
                        # i = c*128 + p:  c in 0..3 -> first-BPE row of word
                        # w = q*512 + p*4 + c;  c in 4..7 -> last-BPE row.
                        t = pool.tile([128, 2 * jj * E], _F32, tag="t")
                        nc.gpsimd.dma_gather(
                            t[:].rearrange("p (c e) -> p c e", e=E),
                            x[:, :],
                            its[q],
                            nidx,
                            nidx,
                            E,
                            queue_num=q % nq,
                        )
                        c = pool.tile([128, jj * E], _F32, tag="c")
                        nc.vector.tensor_add(
                            out=c[:], in0=t[:, : jj * E], in1=t[:, jj * E :]
                        )
                        if not hscale:
                            nc.scalar.mul(c[:], c[:], 0.5)
                    else:
                        # one 2-row window [s, s+2) per word, 6KB descriptors;
                        # out[w] = win[0]*r1 + win[1]*r2 kills the junk row
                        # (m=1: r=(1,0); m=2: r=(.5,.5)).
                        t = pool.tile([128, 2 * jj * E], _F32, tag="t")
                        xw = bass.AP(x.tensor, 0, [[E, ROWS - 1], [1, 2 * E]])
                        nc.gpsimd.dma_gather(
                            t[:].rearrange("p (c e) -> p c e", e=2 * E),
                            xw,
                            its[q],
                            cpw,
                            cpw,
                            2 * E,
                            elem_step=E,
                            queue_num=q % nq,
                        )
                        t3 = t[:].rearrange("p (c e) -> p c e", e=2 * E)
                        r3 = rts[q][:].rearrange("p (c f) -> p c f", f=2)
                        c = pool.tile([128, jj * E], _F32, tag="c")
                        c3 = pool.tile([128, jj * E], _F32, tag="c3")
                        cv = c[:].rearrange("p (j e) -> p j e", e=E)
                        c3v = c3[:].rearrange("p (j e) -> p j e", e=E)
                        nc.vector.tensor_tensor(
                            out=cv,
                            in0=t3[:, :, :E],
                            in1=r3[:, :, 0:1].to_broadcast([128, jj, E]),
                            op=mybir.AluOpType.mult,
                        )
                        nc.vector.tensor_tensor(
                            out=c3v,
                            in0=t3[:, :, E:],
                            in1=r3[:, :, 1:2].to_broadcast([128, jj, E]),
                            op=mybir.AluOpType.mult,
                        )
                        nc.vector.tensor_add(out=c[:], in0=c[:], in1=c3[:])
                    ychunk = y[q * cpw : (q + 1) * cpw, :]
                    if order == "pc":
                        nc.sync.dma_start(
                            out=ychunk.rearrange("(p j) e -> p (j e)", p=128),
                            in_=c[:],
                        )
                    else:
                        nc.sync.dma_start(
                            out=ychunk.rearrange("(j p) e -> p j e", p=128),
                            in_=c[:].rearrange("p (j e) -> p j e", e=E),
                        )
    nc.compile()
    return nc


_NC = {}


def _get_nc(reps=1, bufs=2, order="pc", nq=1, mode="ab", nchunk=NCHUNK,
            merged_idx=False, warm=False, hscale=False):
    key = (reps, bufs, order, nq, mode, nchunk, merged_idx, warm, hscale)
    if key not in _NC:
        _NC[key] = _build_nc(reps, bufs, order, nq, mode, nchunk, merged_idx,
                             warm, hscale)
    return _NC[key]


def _wrap16(flat):
    """int16 index list -> [128, n/16] wrapped (i -> [i%16, i//16]) + 8x rep."""
    return np.tile(flat.reshape(-1, 16).T, (8, 1)).astype(np.int16)


def _make_in_maps(output, mappings, order="pc", mode="ab", nchunk=NCHUNK,
                  hscale=False):
    output = np.asarray(output)
    if hscale:
        # fold the *0.5 of the span mean into the shard upload: a/2 + b/2
        # rounds identically to (a+b)/2 in f32 (halving is exact).
        output = output * np.float32(0.5)
    output = np.ascontiguousarray(output, dtype=np.float32)
    mappings = np.asarray(mappings, dtype=np.int32)
    ends = np.cumsum(mappings, axis=1, dtype=np.int32)  # [B, W] exclusive ends
    src_a = ends - mappings + 1                         # +1: skip [CLS]
    src_b = ends                                        # (e-1) + 1

    in_maps = []
    for k in range(NCORES):
        bs = slice(k * BPC, (k + 1) * BPC)
        base = (np.arange(BPC, dtype=np.int32) * S)[:, None]
        a = (src_a[bs] + base).reshape(-1)
        b = (src_b[bs] + base).reshape(-1)
        pad = np.zeros(NWP - NW, np.int32)
        a = np.concatenate([a, pad])  # [NWP] word-ordered flat row ids
        b = np.concatenate([b, pad])
        x = np.ascontiguousarray(output[bs].reshape(ROWS, E))
        cpw = NWP // nchunk
        jj = cpw // 128
        if mode == "ab":
            idx = np.empty((nchunk, 128, 2 * cpw // 16), np.int16)
            for q in range(nchunk):
                aq = a[q * cpw : (q + 1) * cpw]
                bq = b[q * cpw : (q + 1) * cpw]
                if order == "pc":
                    # gathered i = c*128 + p holds word q*cpw + p*jj + c
                    aq = aq.reshape(128, jj).T.ravel()
                    bq = bq.reshape(128, jj).T.ravel()
                # 'seq': gathered i holds word q*cpw + i (ascending rows)
                idx[q] = _wrap16(np.concatenate([aq, bq]))
            in_maps.append({"x": x, "idx": idx})
        else:
            m = np.concatenate(
                [mappings[bs].reshape(-1), np.ones(NWP - NW, np.int32)]
            ).astype(np.float32)
            r1 = 1.0 / m
            r2 = (m - 1.0) / m
            idx = np.empty((nchunk, 128, cpw // 16), np.int16)
            rw = np.empty((nchunk, 128, 2 * jj), np.float32)
            for q in range(nchunk):
                sl = slice(q * cpw, (q + 1) * cpw)
                aq = a[sl].reshape(128, jj).T.ravel()  # i = c*128 + p
                idx[q] = _wrap16(aq)
                rw[q, :, 0::2] = r1[sl].reshape(128, jj)
                rw[q, :, 1::2] = r2[sl].reshape(128, jj)
            in_maps.append({"x": x, "idx": idx, "rw": rw})
    return in_maps


def _run(output, mappings, reps=1, bufs=2, order="pc", nq=1, mode="ab",
         nchunk=NCHUNK, merged_idx=False, warm=False, hscale=False, **kw):
    in_maps = _make_in_maps(output, mappings, order, mode, nchunk, hscale)
    nc = _get_nc(reps, bufs, order, nq, mode, nchunk, merged_idx, warm, hscale)
    res = run_bass_kernel_spmd(nc, in_maps, list(range(NCORES)), **kw)
    outs = [r["y"][:NW].reshape(BPC, W, E) for r in res.results]
    return np.concatenate(outs, axis=0), res


# Best HW-verified configuration: 8 chunks of 256 words (512-index gathers,
# ~4.6us Q7 descriptor-gen each, pipelined against ~6us transfers), triple
# buffering, single merged index load.
_CFG = dict(bufs=4, order="pc", nq=1, mode="ab", nchunk=8, merged_idx=True)


def kernel(output, mappings):
    full, _ = _run(output, mappings, **_CFG)
    return full
